# revision 65
# baseline (speedup 1.0000x reference)
"""Trainium2 Bass kernel for factorized spatial attention (nn_Attention_50379966382361).

Reference computation (per batch b, frame f):
    qkv = x @ Wqkv.T ; split into q,k,v heads (8 heads, hd=64)
    attn = softmax(q @ k.T * hd**-0.5) over spatial tokens (n=784) within frame
    out  = attn @ v ; merge heads ; y = out @ Wproj.T + bproj

Sharding: data-parallel over the 32 (b, f) frames -> 4 frames per core,
weights replicated.

v3 design -- ACT exp is the bottleneck (~47us/frame), everything else hides:
  - bf16 everywhere except the scores matmul: fp8 noise anywhere on the
    value path (v, e, out, proj) fails the 2e-2 max-abs gate, but fp8 on
    q/k only perturbs logits ~1e-2 relative after softmax.
  - scores use fp8 DoubleRow (0.5 cycles/row, 2x32 k-tiles packed along the
    free dim): q/k are computed in bf16, scaled x16 into fp8 via the
    PSUM->SBUF copy, then re-laid-out to [32, 2, 784] per head (channel
    c -> partition c//2, slot c%2) with partition-crossing SBUF DMAs.
  - attn@v is token-major bf16: out[i, it, 0:64] accumulates in PSUM
    [112, 7, 65] per head, softmax sums in column 64 via ones-matmuls, so
    normalization is a per-partition DVE reciprocal + broadcast multiply.
    All PSUM accumulation groups are emitted as contiguous instruction
    runs: hardware drops a group's partials if another group's
    start_tensor_calc lands in the same bank while it is open.
  - normalized outputs transpose back to channel-major (PE, bf16) for the
    projection; head h-1's attnv drains during head h's scores, per-head
    normalize follows, 2-head transpose slices trail by another head, and
    head 7 + projection cross into the next frame's scores (fine-sliced so
    the in-order PE queue never delays ACT's next exp by much).
"""

import os

import numpy as np
import ml_dtypes

import concourse.bass as bass
import concourse.mybir as mybir
import concourse.tile as tile

B, F, N, VD, D, H = 2, 16, 784, 512, 512, 8
HD = D // H                      # 64
NCORES = 8
FPC = (B * F) // NCORES          # frames per core = 4
JT = 112                         # token tile (7 * 112 = 784, no tail)
NJ = N // JT                     # 7
CHUNKS = ((0, 512), (512, 272))  # free-dim chunks of 784 (PSUM bank = 512 fp32)
FP32 = mybir.dt.float32
BF = mybir.dt.bfloat16
F8 = mybir.dt.float8e4
BFNP = ml_dtypes.bfloat16
AF = mybir.ActivationFunctionType
DR = mybir.MatmulPerfMode.DoubleRow
QSCALE = 16.0                    # q/k fp8 scale (folded into exp scale)
ESCALE = (HD ** -0.5) / (QSCALE * QSCALE)


def _split_ctrl_waits(nc):
    """This walrus build only accepts a single sync-wait per instruction
    (setupSyncWait raises "Too many sync wait commands"), while Tile's
    scheduler aggregates several.  Move the excess waits onto NoOps inserted
    just before (same engine; engines execute in order, so waiting earlier
    on the same queue is equivalent)."""
    for f in nc.m.functions:
        for blk in f.blocks:
            new_list, changed = [], False
            for inst in blk.instructions:
                si = inst.sync_info
                if si is not None and len(si.on_wait) > 1:
                    waits = list(si.on_wait)
                    for w_i, w in enumerate(waits[:-1]):
                        new_list.append(
                            mybir.InstNoOp(
                                name=f"{inst.name}-waitsplit{w_i}",
                                ins=[],
                                outs=[],
                                engine=inst.engine,
                                bass_nofuse=True,
                                sync_info=mybir.SyncInfo(on_wait=[w], on_update=[]),
                            )
                        )
                    inst.sync_info = mybir.SyncInfo(
                        on_wait=[waits[-1]], on_update=list(si.on_update)
                    )
                    changed = True
                new_list.append(inst)
            if changed:
                blk.instructions = new_list


def build_nc():
    nc = bass.Bass("TRN2", target_bir_lowering=False, debug=False, num_devices=NCORES)

    # host pre-arranges everything into on-chip layouts (dense DMAs only)
    xT = nc.declare_dram_parameter("xT", [FPC, 128, 4, N], BF, isOutput=False)
    Wq = nc.declare_dram_parameter("Wq", [128, 4, 3 * D], BF, isOutput=False)
    Wp = nc.declare_dram_parameter("Wp", [128, 4, VD], BF, isOutput=False)
    bproj = nc.declare_dram_parameter("bproj", [VD], FP32, isOutput=False)
    iden = nc.declare_dram_parameter("iden", [JT, JT], BF, isOutput=False)
    yT = nc.declare_dram_parameter("yT", [FPC, VD, N], FP32, isOutput=True)

    with tile.TileContext(nc) as tc:
        with (
            nc.allow_low_precision(reason="bf16/fp8 matmul pipeline"),
            tc.tile_pool(name="w", bufs=1) as w_pool,
            tc.tile_pool(name="x", bufs=3) as x_pool,
            tc.tile_pool(name="qkc", bufs=3) as qkc_pool,
            tc.tile_pool(name="qk", bufs=32) as qk_pool,
            tc.tile_pool(name="v", bufs=8) as v_pool,
            tc.tile_pool(name="e", bufs=20) as e_pool,
            tc.tile_pool(name="r", bufs=2) as r_pool,
            tc.tile_pool(name="ot", bufs=2) as ot_pool,
            tc.tile_pool(name="xp", bufs=2) as xp_pool,
            tc.tile_pool(name="y", bufs=3) as y_pool,
            tc.tile_pool(name="mm", bufs=2, space="PSUM") as mm_ps,
            tc.tile_pool(name="av", bufs=4, space="PSUM") as av_ps,
        ):
            # ---- constants / weights (once per core) ----
            W1 = w_pool.tile([128, 4, 3 * D], BF)
            for kt in range(4):
                nc.gpsimd.dma_start(out=W1[:, kt], in_=Wq[:, kt])
            W2 = w_pool.tile([128, 4, VD], BF)
            nc.gpsimd.dma_start(out=W2, in_=Wp[:])
            bias_sb = w_pool.tile([128, 4], FP32)
            nc.scalar.dma_start(out=bias_sb, in_=bproj.rearrange("(a p) -> p a", p=128))
            idt = w_pool.tile([JT, JT], BF)
            nc.scalar.dma_start(out=idt, in_=iden[:])
            ones1 = w_pool.tile([JT, 1], BF)
            nc.vector.memset(ones1, 1.0)
            # dummy exp pulls the one-time ACT table load into the DMA wait
            warm_f = w_pool.tile([1, 8], FP32)
            nc.vector.memset(warm_f, 0.0)
            warm = w_pool.tile([1, 8], FP32)
            nc.scalar.activation(out=warm, in_=warm_f, func=AF.Exp)
            # PE warmup: ~6us of dummy matmuls during the initial DMA waits
            # ramp the tensor engine to full p-state before frame 0's qkv
            wsrc = w_pool.tile([128, 512], BF)
            nc.vector.memset(wsrc, 0.0)
            for wi in range(14):
                ps_w = av_ps.tile([128, 512], FP32, tag="av")
                nc.tensor.matmul(ps_w, wsrc[:, 0:128], wsrc,
                                 start=True, stop=True)

            reps = int(os.environ.get("KERNEL_TIME_REPS", "1"))
            frames = [f for _ in range(reps) for f in range(FPC)]

            # deferred cross-frame work, drained in small slices during the
            # next frame's scores so ACT never waits long on PE
            pending = []

            def drain(k):
                # pending items may be closures or generators (fine-sliced)
                done = 0
                while done < k and pending:
                    item = pending[0]
                    if not hasattr(item, "__next__"):
                        res = item()
                        if res is None:
                            pending.pop(0)
                            done += 1
                            continue
                        pending[0] = item = res
                    try:
                        next(item)
                        done += 1
                    except StopIteration:
                        pending.pop(0)

            def gen_head(fr, first=False):
                """Stage-A front for frame fr (loads, q/k+rearrange, v), as
                a generator interleaved into the previous frame's scores.
                PSUM comes from the 1-bank av pool so the scores (mm) ring
                never blocks on it.  head_state fills incrementally; with
                first=True head 0's q/k tiles are emitted before v so the
                first frame can start scoring after two steps."""
                qk32, v8 = {}, []
                head_state[fr] = (qk32, v8)
                v_done[fr] = False
                X8 = x_pool.tile([128, 4, N], BF, tag="X")
                nc.sync.dma_start(out=X8, in_=xT[fr])

                def emit_qk_tile(ot):
                    # yields between kt-pairs so each consumed step adds
                    # <~900ns to the in-order PE queue
                    t8 = qkc_pool.tile([128, N], F8, tag="qkc")
                    for c0, cw in CHUNKS:
                        ps = av_ps.tile([128, cw], FP32, tag="av")
                        for kt in range(4):
                            nc.tensor.matmul(
                                ps,
                                W1[:, kt, ot * 128 : (ot + 1) * 128],
                                X8[:, kt, c0 : c0 + cw],
                                start=(kt == 0),
                                stop=(kt == 3),
                            )
                            if kt == 1 and cw == 512:
                                yield
                        nc.vector.tensor_scalar(
                            out=t8[:, c0 : c0 + cw],
                            in0=ps,
                            scalar1=QSCALE,
                            scalar2=None,
                            op0=mybir.AluOpType.mult,
                        )
                        yield
                    for half in range(2):
                        h32 = qk_pool.tile([32, 2, N], F8, tag="qk32")
                        nc.sync.dma_start(
                            out=h32, in_=t8[64 * half : 64 * half + 64, :]
                        )
                        qk32[(ot % 4) * 2 + half, ot < 4] = h32

                if first:  # head 0 can start scoring quickly
                    yield from emit_qk_tile(0)
                    yield from emit_qk_tile(4)

                # v before the remaining q/k: all v8 tiles must be emitted
                # before any attnv of this frame (head 1 of its scores)
                for _ in range((NJ + 1) // 2):
                    vt = v_pool.tile([JT, 2, H, HD + 1], BF, tag="v8")
                    nc.gpsimd.memset(vt[:, :, :, HD], 1.0)
                    v8.append(vt)
                for tt in range(NJ):
                    psv = av_ps.tile([JT, D], FP32, tag="av")
                    for kt in range(4):
                        nc.tensor.matmul(
                            psv,
                            X8[:, kt, tt * JT : (tt + 1) * JT],
                            W1[:, kt, 2 * D : 3 * D],
                            start=(kt == 0),
                            stop=(kt == 3),
                        )
                        if kt == 1:
                            yield
                    nc.vector.tensor_copy(
                        v8[tt // 2][:, tt % 2, :, 0:HD],
                        psv.rearrange("p (h c) -> p h c", c=HD),
                    )
                    yield
                v_done[fr] = True

                for ot in (0, 4, 1, 5, 2, 6, 3, 7):  # q tile then its k tile
                    if (ot in (0, 4)) and first:
                        continue
                    yield from emit_qk_tile(ot)

            head_state = {}
            v_done = {}

            def emit_frame(fr, gens, e0_tiles, last=False, first=False):
                qk32, v8 = head_state[fr]
                r8 = r_pool.tile([JT, NJ, H], FP32, tag="r8")
                otok = ot_pool.tile([JT, NJ, H, HD], BF, tag="otok")
                xp2 = xp_pool.tile([128, 4, N], BF, tag="xp2")
                e_tiles = dict(e0_tiles)
                avh = {}

                def step_gen():
                    while gens:
                        try:
                            next(gens[0])
                            return
                        except StopIteration:
                            gens.pop(0)

                def do_attnv(h, it):
                    assert v_done[fr], "v8 writes must be emitted first"
                    # avh[h] holds out[i, it, 0:64] + softmax sums in col 64.
                    # Each accumulation group is a CONTIGUOUS instruction
                    # run (av x7 then sums x7): an alien start_tensor_calc
                    # in the same PSUM bank drops an open group's partials.
                    if h not in avh:
                        ah = av_ps.tile([JT, NJ, HD + 1], FP32, tag="av")
                        avh[h] = ah
                    av = avh[h]
                    for jt in range(NJ):
                        nc.tensor.matmul(
                            av[:, it, :],
                            e_tiles[h, jt // 2][:, jt % 2, it * JT : (it + 1) * JT],
                            v8[jt // 2][:, jt % 2, h, :],
                            start=(jt == 0),
                            stop=(jt == NJ - 1),
                        )

                def do_norm(h):
                    av = avh.pop(h)
                    nc.vector.reciprocal(out=r8[:, :, h], in_=av[:, :, HD])
                    nc.vector.tensor_mul(
                        otok[:, :, h, :],
                        av[:, :, 0:HD],
                        r8[:, :, h].unsqueeze(2).broadcast_to([JT, NJ, HD]),
                    )

                def do_trslice(dt_):
                    # transpose heads (2dt, 2dt+1) of all i-tiles back to
                    # channel-major
                    tr = av_ps.tile([128, NJ, JT], BF, tag="av")
                    for it in range(NJ):
                        nc.tensor.transpose(
                            out=tr[:, it, :],
                            in_=otok[:, it, 2 * dt_ : 2 * dt_ + 2, :]
                            .rearrange("p h c -> p (h c)"),
                            identity=idt,
                        )
                    nc.vector.tensor_copy(xp2[:, dt_, :], tr)

                def do_proj(ot):
                    yt = y_pool.tile([128, N], FP32, tag="yT")
                    if last:
                        # scores are done: idle 2-bank mm pool + idle ACT
                        ps_f = mm_ps.tile([128, N], FP32, tag="mm")
                        for c0, cw in CHUNKS:
                            for kt in range(4):
                                nc.tensor.matmul(
                                    ps_f[:, c0 : c0 + cw],
                                    W2[:, kt, ot * 128 : (ot + 1) * 128],
                                    xp2[:, kt, c0 : c0 + cw],
                                    start=(kt == 0),
                                    stop=(kt == 3),
                                )
                        nc.scalar.activation(
                            out=yt,
                            in_=ps_f,
                            func=AF.Identity,
                            bias=bias_sb[:, ot : ot + 1],
                        )
                    else:
                        for c0, cw in CHUNKS:
                            ps_y = av_ps.tile([128, cw], FP32, tag="av")
                            for kt in range(4):
                                nc.tensor.matmul(
                                    ps_y,
                                    W2[:, kt, ot * 128 : (ot + 1) * 128],
                                    xp2[:, kt, c0 : c0 + cw],
                                    start=(kt == 0),
                                    stop=(kt == 3),
                                )
                                if kt == 1 and cw == 512:
                                    yield
                            nc.vector.tensor_scalar(
                                out=yt[:, c0 : c0 + cw],
                                in0=ps_y,
                                scalar1=1.0,
                                scalar2=bias_sb[:, ot : ot + 1],
                                op0=mybir.AluOpType.mult,
                                op1=mybir.AluOpType.add,
                            )
                            yield
                    nc.sync.dma_start(
                        out=yT[fr, ot * 128 : (ot + 1) * 128, :], in_=yt
                    )

                for h in range(1, H):
                    qh = qk32[h, True]
                    kh = qk32[h, False]
                    for jt in range(NJ):
                        if jt % 2 == 0:
                            ep = e_pool.tile([JT, 2, N], BF, tag="e2")
                            e_tiles[h, jt // 2] = ep
                        ps = mm_ps.tile([JT, N], FP32, tag="mm")
                        for c0, cw in CHUNKS:
                            nc.tensor.matmul(
                                ps[:, c0 : c0 + cw],
                                kh[:, :, jt * JT : (jt + 1) * JT],
                                qh[:, :, c0 : c0 + cw],
                                start=True,
                                stop=True,
                                perf_mode=DR,
                            )
                        nc.scalar.activation(
                            out=e_tiles[h, jt // 2][:, jt % 2],
                            in_=ps,
                            func=AF.Exp,
                            scale=ESCALE,
                        )
                        # fine-grained deferred work between exps
                        if h >= 1:
                            do_attnv(h - 1, jt)
                        if h <= 3:
                            drain(1)
                        step_gen()
                    if h >= 1:
                        do_norm(h - 1)
                    if h in (4, 6):
                        do_trslice((h - 4) // 2)
                if last:
                    for it in range(NJ):
                        do_attnv(7, it)
                    do_norm(7)
                    do_trslice(2)
                    do_trslice(3)
                    for ot in range(4):
                        for _ in do_proj(ot):
                            pass
                else:
                    # head 7 + frame epilogue: the next frame's head-0
                    # scores run here, their windows draining head 7's
                    # attnv so the boundary looks like any head boundary
                    do_trslice(2)
                    for it in range(NJ):
                        pending.append(lambda it=it: do_attnv(7, it))
                    pending.append(lambda: do_norm(7))
                    pending.append(lambda: do_trslice(3))
                    for ot in range(4):
                        pending.append(lambda ot=ot: do_proj(ot))

            def emit_h0(fr, triple=False):
                qk32, _ = head_state[fr]
                qh = qk32[0, True]
                kh = qk32[0, False]
                e0 = {}
                for jt in range(NJ):
                    if jt % 2 == 0:
                        ep = e_pool.tile([JT, 2, N], BF, tag="e2")
                        e0[0, jt // 2] = ep
                    ps = mm_ps.tile([JT, N], FP32, tag="mm")
                    for c0, cw in CHUNKS:
                        nc.tensor.matmul(
                            ps[:, c0 : c0 + cw],
                            kh[:, :, jt * JT : (jt + 1) * JT],
                            qh[:, :, c0 : c0 + cw],
                            start=True,
                            stop=True,
                            perf_mode=DR,
                        )
                    nc.scalar.activation(
                        out=e0[0, jt // 2][:, jt % 2],
                        in_=ps,
                        func=AF.Exp,
                        scale=ESCALE,
                    )
                    drain(1)
                    step_gen_outer()
                    if triple:
                        step_gen_outer()
                        step_gen_outer()
                return e0

            def step_gen_outer():
                while gens:
                    try:
                        next(gens[0])
                        return
                    except StopIteration:
                        gens.pop(0)

            gen0 = gen_head(frames[0], first=True)
            next(gen0)  # prime: head_state entry + X load
            # pre-consume only head-0's q/k tiles; v interleaves into head
            # 0's scores at triple pace (21 steps over 7 jts), finishing
            # exactly before head 1's attnv needs it (asserted in do_attnv)
            while (0, False) not in head_state[frames[0]][0]:
                next(gen0)
            gens = [gen0]
            e0 = emit_h0(frames[0], triple=True)
            for idx, fr in enumerate(frames):
                if idx + 1 < len(frames):
                    gens.append(gen_head(frames[idx + 1]))
                emit_frame(fr, gens, e0,
                           last=(idx + 1 == len(frames)), first=(idx == 0))
                if idx + 1 < len(frames):
                    e0 = emit_h0(frames[idx + 1])
                while gens:  # exhaust any leftover stage-A work
                    try:
                        next(gens[0])
                    except StopIteration:
                        gens.pop(0)
            drain(len(pending))

    _split_ctrl_waits(nc)
    return nc


_CACHE = {}


def _get_runner():
    """Build the Bass module once and wrap it in a cached sharded jax.jit
    callable (replicates bass2jax.run_bass_via_pjrt but reusable across
    calls, so repeated invocations don't re-lower/re-compile)."""
    if "runner" in _CACHE:
        return _CACHE["runner"]

    import jax
    from jax.experimental.shard_map import shard_map
    from jax.sharding import Mesh, PartitionSpec
    from concourse import bass2jax, mybir as _mybir

    nc = build_nc()
    bass2jax.install_neuronx_cc_hook()
    assert nc.dbg_addr is None
    partition_name = nc.partition_id_tensor.name if nc.partition_id_tensor else None

    in_names, out_names, out_avals, out_shapes = [], [], [], []
    for alloc in nc.m.functions[0].allocations:
        if not isinstance(alloc, _mybir.MemoryLocationSet):
            continue
        name = alloc.memorylocations[0].name
        if alloc.kind == "ExternalInput":
            if name != partition_name:
                in_names.append(name)
        elif alloc.kind == "ExternalOutput":
            shape = tuple(alloc.tensor_shape)
            dtype = _mybir.dt.np(alloc.dtype)
            out_names.append(name)
            out_avals.append(jax.core.ShapedArray(shape, dtype))
            out_shapes.append((shape, dtype))
    n_params = len(in_names)
    all_names = in_names + out_names
    if partition_name is not None:
        all_names = all_names + [partition_name]

    def _body(*args):
        operands = list(args)
        if partition_name is not None:
            operands.append(bass2jax.partition_id_tensor())
        outs = bass2jax._bass_exec_p.bind(
            *operands,
            out_avals=tuple(out_avals),
            in_names=tuple(all_names),
            out_names=tuple(out_names),
            lowering_input_output_aliases=(),
            sim_require_finite=True,
            sim_require_nnan=True,
            nc=nc,
        )
        return tuple(outs)

    devices = jax.devices()[:NCORES]
    mesh = Mesh(np.asarray(devices), ("core",))
    nin = n_params + len(out_names)
    sharded = jax.jit(
        shard_map(
            _body,
            mesh=mesh,
            in_specs=(PartitionSpec("core"),) * nin,
            out_specs=(PartitionSpec("core"),) * len(out_names),
            check_rep=False,
        ),
        donate_argnums=tuple(range(n_params, nin)),
        keep_unused=True,
    )

    def run(in_maps):
        concat_in = [
            np.concatenate([np.asarray(m[name]) for m in in_maps], axis=0)
            for name in in_names
        ]
        last_err = None
        for attempt in range(3):
            # fresh zeros each attempt (donated buffers are consumed even on
            # a failed dispatch)
            concat_zeros = [
                np.zeros((NCORES * s[0], *s[1:]), dt) for s, dt in out_shapes
            ]
            try:
                out_arrs = sharded(*concat_in, *concat_zeros)
                # materialize inside the retry scope: transient device errors
                # (e.g. NRT_EXEC_UNIT_UNRECOVERABLE through the axon relay)
                # can surface at fetch time
                host = [np.asarray(o) for o in out_arrs]
                return [
                    {
                        name: host[i].reshape(NCORES, *out_shapes[i][0])[c]
                        for i, name in enumerate(out_names)
                    }
                    for c in range(NCORES)
                ]
            except Exception as e:  # noqa: BLE001 - retry transient device faults
                last_err = e
                import time as _time

                _time.sleep(2.0 * (attempt + 1))
        raise last_err

    _CACHE["runner"] = run
    _CACHE["parts"] = dict(
        nc=nc, sharded=sharded, in_names=in_names, out_names=out_names,
        out_shapes=out_shapes, mesh=mesh, n_params=n_params,
    )
    return run


def prepare_in_maps(x, Wqkv, Wproj, bproj):
    x = np.ascontiguousarray(np.asarray(x, dtype=np.float32))
    Wqkv = np.asarray(Wqkv, dtype=np.float32)
    Wproj = np.asarray(Wproj, dtype=np.float32)
    bp = np.ascontiguousarray(np.asarray(bproj, dtype=np.float32))

    # x: (b, f*n, d) -> (b*f, p, kt, n) with channel d = 128*kt + p
    xt = np.ascontiguousarray(
        x.reshape(B * F, N, 4, 128).transpose(0, 3, 2, 1).astype(BFNP)
    )
    # W [m, d_in] -> [p, kt, m]
    Wqh = np.ascontiguousarray(
        Wqkv.T.reshape(4, 128, 3 * D).transpose(1, 0, 2).astype(BFNP)
    )
    Wph = np.ascontiguousarray(
        Wproj.T.reshape(4, 128, VD).transpose(1, 0, 2).astype(BFNP)
    )
    iden = np.eye(JT, dtype=np.float32).astype(BFNP)
    return [
        {
            "xT": np.ascontiguousarray(xt[c * FPC : (c + 1) * FPC]),
            "Wq": Wqh,
            "Wp": Wph,
            "bproj": bp,
            "iden": iden,
        }
        for c in range(NCORES)
    ]


def kernel(x, Wqkv, Wproj, bproj, spatial=None, f=None, n=None, **_ignored):
    in_maps = prepare_in_maps(x, Wqkv, Wproj, bproj)
    results = _get_runner()(in_maps)

    y = np.empty((B * F, N, VD), dtype=np.float32)
    for c in range(NCORES):
        y[c * FPC : (c + 1) * FPC] = results[c]["yT"].transpose(0, 2, 1)
    return y.reshape(B, F * N, VD)
